# revision 12
# baseline (speedup 1.0000x reference)
"""Trainium2 Bass kernel for nn_Attention_32650341384246.

Full attention layer: qkv proj + per-head RMSNorm(q,k) + RoPE + softmax
attention (non-causal) + out proj.  B=2, S=2048, D=1024, H=16, DH=64.

Sharding: 8 cores; core c handles batch c//4, heads [4*(c%4), 4*(c%4)+4)
(data parallel over batch x tensor parallel over heads).  Each core
computes a partial output (its heads @ Wout row-slice) TRANSPOSED
[DM, S]; the host sums the 4 partials per batch and adds folded biases.

Device design (per core), tuned for engine balance
(PE ~saturated; ACT carries all softmax exp; DVE the elementwise rest):
  - qkv proj emits qT/kT head-major [128 (2 heads x 64), S] (lhsT = W
    slice, rhs = xT slice) and v s-major [s, 4*64].
  - bias-add (tensor_scalar) + square (custom SQBIAS) on DVE; sumsq via
    ones-block matmul (bf16); rsqrt via 2-inst custom DVE chain
    (deg-3 Horner seed + fitted Newton step) -- no ACT Ln/Exp, so the
    whole kernel uses ONE activation table set (exp_and_others).
  - RoPE as q_rot = cosT*u + sinT'*swap(u); swap = adjacent-partition
    permutation matmul (bf16, exact); cos/sin tables bf16 with
    q_scale/k_scale folded in.
  - scores^T [k, q] bf16 matmuls per 512-wide q chunk (K=64), two heads
    on distinct PE row groups with forced emission adjacency so they run
    CONCURRENTLY; score PSUM double-buffered so sc(kt+1) overlaps exp(kt).
  - softmax exp entirely on ACT (PSUM -> bf16), 1/sqrt(dh) via exp scale.
  - AV via lhsT = [v | ones] bf16 (M=65): row 64 accumulates sumexp;
    h-outer/qv-inner order shares LDWEIGHTS between consecutive MMs.
  - normalize: gather 4 sumexp rows -> one DVE reciprocal -> bf16
    select-matrix matmul broadcasts reciprocal rows across partitions.
  - out proj TRANSPOSED: lhsT = Wout chunk (stationary across 4 s-chunks
    -> LDW amortized), rhs = vmix; psum -> bf16 copies -> DMA [DM, S].
"""
import sys, os

sys.path.insert(0, "/opt/trn_rl_repo")

import numpy as np
from contextlib import ExitStack

import ml_dtypes
import concourse.bass as bass
import concourse.mybir as mybir
import concourse.tile as tile
from concourse import bacc
from concourse import bass_utils
import concourse.dve_ops as dve_ops
from concourse.dve_ops import (DveOp, RECIPROCAL_APPROX_FAST,
                               RECIP_APPROX_FAST_CONSTS)
from concourse.dve_spec import (
    Spec, Src0, Src1, C0, C1, C2, C3, lower, _spill_c3_to_src1,
    _has_src1 as _has_src1,
)
from concourse.dve_uop import DveOpSpec

F32 = mybir.dt.float32
F32R = mybir.dt.float32r
BF16 = mybir.dt.bfloat16
AF = mybir.ActivationFunctionType

B, S, DM, H, DH = 2, 2048, 1024, 16, 64
NC = 8
HPC = H // 4          # 4 heads per core
HD = HPC * DH         # 256
NDT = DM // 128       # 8 model-dim tiles
THETA, EPS = 10000.0, 1e-6

LAST_RESULTS = None   # BassKernelResults of the most recent device run
_CACHED = {}

# knobs
T1_ON_GPSIMD = True    # u*cos and t1+t2 (SBUF-only) on GpSimd
PO_COPY_ACT = 16       # of 32 phase-3 psum->sbuf copies on ACT (rest DVE)

# ---------------- custom DVE ops (registered at import) ----------------


def _register_dve_op(name, spec, subdim=False):
    if name in dve_ops._SUB_OPCODE_FOR_NAME:
        for op in dve_ops.OPS:
            if op.name == name:
                return op
        raise RuntimeError(f"{name} registered but not in OPS")
    row = dve_ops._CUSTOM_DVE_ROW_BASE + len(dve_ops.OPS)
    assert row < 0x20, "custom DVE op rows exhausted"
    dve_ops._SUB_OPCODE_FOR_NAME[name] = row
    shas = {"v3": DveOpSpec(name=name, opcode=row,
                            uops=lower(spec, ver="v3"),
                            rd1_en=_has_src1(spec)).sha("v3")}
    op = DveOp(name, spec, subdim=subdim, uops_sha=shas)
    dve_ops.OPS.append(op)
    dve_ops.CUSTOM_DVE_SPECS[name] = spec
    return op


# sq = (x + b)^2 with per-partition b; reads score PSUM once.
_sb = Src0 + C0
SQBIAS = _register_dve_op(
    "SQBIAS_ANT",
    Spec(body=_sb * _sb,
         reference=lambda in0, in1, s0, s1, imm2: (
             (np.asarray(in0, np.float32) + np.float32(s0)) ** 2
         ).astype(np.float32)))

# rsqrt(2m) over m in [0.052, 0.385]: deg-3 Horner seed ...
RSQ_C3 = -88.55851031561393
RSQ_C2 = 78.57457530349905
RSQ_C1 = -24.768702251743473
RSQ_C0 = 4.054988803119327   # via in1 [P,1]
_s1 = Src0 * C0
_s2 = _s1 + C1
_s3 = _s2 * Src0
_s4 = _s3 + C2
_s5 = _s4 * Src0
_seed_body = _spill_c3_to_src1(_s5 + C3)


def _ref_rsq_seed(in0, in1, s0, s1, imm2):
    m = np.asarray(in0, np.float32)
    c0 = np.asarray(in1, np.float32).reshape(m.shape[0], 1)
    t = (((m * np.float32(s0) + np.float32(s1)) * m + np.float32(imm2)) * m
         + c0)
    return t.astype(np.float32)


RSQ_SEED = _register_dve_op("RSQ_SEED_ANT",
                            Spec(body=_seed_body, reference=_ref_rsq_seed))

# ... then fitted Newton y1 = q*(A - B*m*q^2), q via in1.
RSQ_NA = 1.518420851483698
RSQ_NB = 1.035705175407688
_q2 = Src1 * Src1
_h = _q2 * Src0
_hb = _h * C0
_w = C1 - _hb
_newt_body = Src1 * _w


def _ref_rsq_newt(in0, in1, s0, s1, imm2):
    m = np.asarray(in0, np.float32)
    q = np.asarray(in1, np.float32)
    w = (np.float32(s1) - q * q * m * np.float32(s0)).astype(np.float32)
    return (q * w).astype(np.float32)


RSQ_NEWT = _register_dve_op("RSQ_NEWT_ANT",
                            Spec(body=_newt_body, reference=_ref_rsq_newt))


def build_program(exp_scale: float, shared_tables: bool):
    nc = bacc.Bacc("TRN2", target_bir_lowering=False, debug=False)

    xT_d = nc.dram_tensor("xT", [128, NDT, S], BF16, kind="ExternalInput")
    w_d = nc.dram_tensor("w_all", [128, NDT, 3 * HD], BF16, kind="ExternalInput")
    wout_d = nc.dram_tensor("wout", [128, 2, DM], BF16, kind="ExternalInput")
    bq_d = nc.dram_tensor("bq", [128, 2], F32, kind="ExternalInput")
    bk_d = nc.dram_tensor("bk", [128, 2], F32, kind="ExternalInput")
    cosk_d = nc.dram_tensor("cos_k", [128, S], BF16, kind="ExternalInput")
    sink_d = nc.dram_tensor("sin_k", [128, S], BF16, kind="ExternalInput")
    if not shared_tables:
        cosq_d = nc.dram_tensor("cos_q", [128, S], BF16, kind="ExternalInput")
        sinq_d = nc.dram_tensor("sin_q", [128, S], BF16, kind="ExternalInput")
    P_d = nc.dram_tensor("Pswap", [128, 128], BF16, kind="ExternalInput")
    ob_d = nc.dram_tensor("onesblk", [128, 2], BF16, kind="ExternalInput")
    o2_d = nc.dram_tensor("ones2blk", [2, 128], BF16, kind="ExternalInput")
    sel_d = nc.dram_tensor("sel", [128, 2, 128], BF16, kind="ExternalInput")
    out_d = nc.dram_tensor("outp", [DM, S], BF16, kind="ExternalOutput")

    with tile.TileContext(nc) as tc, ExitStack() as ctx, \
            nc.allow_low_precision(reason="fp32r/bf16 matmul inputs"):
        singles = ctx.enter_context(tc.tile_pool(name="singles", bufs=1))
        tmp = ctx.enter_context(tc.tile_pool(name="tmp", bufs=2))
        expp = ctx.enter_context(tc.tile_pool(name="expp", bufs=2))
        outp = ctx.enter_context(tc.tile_pool(name="outp", bufs=2))

        w_dt = [singles.tile([128, 3 * HD], BF16, name=f"w{dt}") for dt in range(NDT)]
        x_dt = [singles.tile([128, S], BF16, name=f"x{dt}") for dt in range(NDT)]
        for dt in range(NDT):
            nc.sync.dma_start(out=w_dt[dt], in_=w_d.ap()[:, dt, :])
            nc.sync.dma_start(out=x_dt[dt], in_=xT_d.ap()[:, dt, :])

        wout = singles.tile([128, 2, DM], BF16)
        nc.sync.dma_start(out=wout, in_=wout_d.ap())
        bq = singles.tile([128, 2], F32)
        nc.sync.dma_start(out=bq, in_=bq_d.ap())
        bk = singles.tile([128, 2], F32)
        nc.sync.dma_start(out=bk, in_=bk_d.ap())
        cos_k = singles.tile([128, S], BF16)
        nc.sync.dma_start(out=cos_k, in_=cosk_d.ap())
        sin_k = singles.tile([128, S], BF16)
        nc.sync.dma_start(out=sin_k, in_=sink_d.ap())
        if shared_tables:
            cos_q, sin_q = cos_k, sin_k
        else:
            cos_q = singles.tile([128, S], BF16)
            nc.sync.dma_start(out=cos_q, in_=cosq_d.ap())
            sin_q = singles.tile([128, S], BF16)
            nc.sync.dma_start(out=sin_q, in_=sinq_d.ap())
        Pm = singles.tile([128, 128], BF16)
        nc.sync.dma_start(out=Pm, in_=P_d.ap())
        onesblk = singles.tile([128, 2], BF16)
        nc.sync.dma_start(out=onesblk, in_=ob_d.ap())
        ones2blk = singles.tile([2, 128], BF16)
        nc.sync.dma_start(out=ones2blk, in_=o2_d.ap())
        sel = singles.tile([128, 2, 128], BF16)
        nc.sync.dma_start(out=sel, in_=sel_d.ap())
        c0t = singles.tile([128, 1], F32)
        nc.vector.memset(c0t, RSQ_C0)

        qt = [singles.tile([128, S], BF16, name=f"qt{t}") for t in range(2)]
        kt_ = [singles.tile([128, S], BF16, name=f"kt{t}") for t in range(2)]
        vhat = singles.tile([128, 16, HPC, 65], BF16, name="vhat")
        nc.vector.memset(vhat[:, :, :, 64:65], 1.0)
        vmix = [singles.tile([128, S], BF16, name=f"vmix{t}") for t in range(2)]
        se = singles.tile([128, 512], F32, name="se")
        nc.vector.memset(se, 1.0)

        # ---------------- phase 1: qkv + rmsnorm + rope ----------------
        with tc.tile_pool(name="ps1", bufs=1, space="PSUM") as ps1:
            sections = (
                    ("k", 0, bk, cos_k, sin_k, kt_),
                    ("q", 0, bq, cos_q, sin_q, qt),
                    ("k", 1, bk, cos_k, sin_k, kt_),
                    ("q", 1, bq, cos_q, sin_q, qt))
            for which, t, bias, cosT, sinT, dest in sections:
                off = (0 if which == "q" else HD) + t * 128
                for sc in range(4):       # s-chunks of 512
                    s0 = sc * 512
                    pq = ps1.tile([128, 512], F32, tag="pq", bufs=2,
                                  name=f"pq{which}{t}_{sc}")
                    for dt in range(NDT):
                        nc.tensor.matmul(
                            pq[:, :],
                            w_dt[dt][:, off: off + 128],
                            x_dt[dt][:, s0:s0 + 512],
                            start=(dt == 0), stop=(dt == NDT - 1))
                    tt = tmp.tile([128, 512], F32, tag="tt", bufs=4,
                                  name=f"tt{which}{t}_{sc}")
                    nc.scalar.activation(tt[:, :], pq[:, :], AF.Identity,
                                         bias=bias[:, t:t + 1], scale=1.0)
                    sq = tmp.tile([128, 512], BF16, tag="sq", name=f"sq{which}{t}_{sc}")
                    nc.scalar.activation(sq[:, :], pq[:, :], AF.Square,
                                         bias=bias[:, t:t + 1], scale=1.0)
                    pss = ps1.tile([2, 512], F32, tag="pss", bufs=2,
                                   name=f"pss{which}{t}_{sc}")
                    nc.tensor.matmul(pss[:, :], onesblk[:, :], sq[:, :],
                                     start=True, stop=True)
                    seed = tmp.tile([2, 512], F32, tag="seed", name=f"sd{which}{t}_{sc}")
                    nc.vector._custom_dve(RSQ_SEED, out=seed[:, :], in0=pss[:, :],
                                          in1=c0t[0:2, 0:1],
                                          s0=RSQ_C3, s1=RSQ_C2, imm2=RSQ_C1)
                    rs = tmp.tile([2, 512], BF16, tag="rs", name=f"rs{which}{t}_{sc}")
                    nc.vector._custom_dve(RSQ_NEWT, out=rs[:, :], in0=pss[:, :],
                                          in1=seed[:, :],
                                          s0=RSQ_NB, s1=RSQ_NA)
                    pb = ps1.tile([128, 512], F32, tag="pb",
                                  name=f"pb{which}{t}_{sc}")
                    nc.tensor.matmul(pb[:, :], ones2blk[:, :], rs[:, :],
                                     start=True, stop=True)
                    u = tmp.tile([128, 512], BF16, tag="u", name=f"u{which}{t}_{sc}")
                    nc.vector.tensor_mul(u[:, :], tt[:, :], pb[:, :])
                    psw = ps1.tile([128, 512], F32, tag="psw",
                                   name=f"psw{which}{t}_{sc}")
                    nc.tensor.matmul(psw[:, :], Pm[:, :], u[:, :],
                                     start=True, stop=True)
                    t1 = tmp.tile([128, 512], BF16, tag="t1", name=f"t1{which}{t}_{sc}")
                    eng1 = nc.gpsimd if T1_ON_GPSIMD else nc.vector
                    eng1.tensor_mul(t1[:, :], u[:, :], cosT[:, s0:s0 + 512])
                    t2 = tmp.tile([128, 512], BF16, tag="t2", name=f"t2{which}{t}_{sc}")
                    nc.vector.tensor_mul(t2[:, :], psw[:, :], sinT[:, s0:s0 + 512])
                    eng1.tensor_add(dest[t][:, s0:s0 + 512], t1[:, :], t2[:, :])

            # v section: s-major [s, 4*64] + ones column
            for kt in range(16):
                pv = ps1.tile([128, HD], F32, tag="pv", bufs=2, name=f"pv{kt}")
                for dt in range(NDT):
                    nc.tensor.matmul(
                        pv[:, :],
                        x_dt[dt][:, kt * 128: (kt + 1) * 128],
                        w_dt[dt][:, 2 * HD:3 * HD],
                        start=(dt == 0), stop=(dt == NDT - 1))
                nc.vector.tensor_copy(vhat[:, kt, :, 0:64],
                                      pv[:, :].rearrange("p (h d) -> p h d", h=HPC))

        # ---------------- phase 2: attention ----------------
        # (pair, q-half-1024) groups.  Scores for both heads accumulate in
        # ONE [128, 2, 1024] PSUM tile (4 banks) so softmax exp is a single
        # 2048-wide ACT instruction per kt (amortizes ACT startup; ACT is
        # the phase-2 bottleneck).  av(kt-1) is emitted after sc(kt)/exp(kt)
        # so the PE never head-of-line blocks on the exp latency.
        with tc.tile_pool(name="ps2", bufs=1, space="PSUM") as ps2:
            for pair in range(2):
                for qh in range(2):
                    q0 = qh * 1024
                    avp = [[ps2.tile([65, 512], F32, tag=f"av{h}{c}",
                                     name=f"av{pair}{qh}{h}{c}")
                            for c in range(2)] for h in range(2)]
                    # software-pipelined kt loop; the h1 stream lags h0 by
                    # one kt so sc_h0(kt) and sc_h1(kt-1) are both dep-free
                    # at window start and run as a concurrent row-group pair.
                    # av(kt-1) MMs fill the PE while exp(kt) occupies ACT.
                    pend_av = [None, None]

                    def emit_av(h):
                        pkt, pe_ = pend_av[h]
                        for c in range(2):
                            nc.tensor.matmul(
                                avp[h][c][:, :],
                                vhat[:, pkt, 2 * pair + h, :],
                                pe_[:, c * 512:(c + 1) * 512],
                                start=(pkt == 0), stop=(pkt == 15),
                                skip_group_check=True)
                        pend_av[h] = None

                    for step in range(17):
                        ktq = {0: step, 1: step - 1}
                        pa = {}
                        for h in (0, 1):
                            if 0 <= ktq[h] < 16:
                                pa[h] = ps2.tile([128, 1024], F32, tag=f"sc{h}",
                                                 name=f"sc{pair}{qh}{h}_{ktq[h]}")
                        chain = None
                        for c in range(2):
                            for h in (0, 1):
                                if h not in pa:
                                    continue
                                k = ktq[h]
                                mm = nc.tensor.matmul(
                                    pa[h][:, c * 512:(c + 1) * 512],
                                    kt_[pair][h * 64:(h + 1) * 64,
                                              k * 128:(k + 1) * 128],
                                    qt[pair][h * 64:(h + 1) * 64,
                                             q0 + c * 512:q0 + (c + 1) * 512],
                                    start=True, stop=True,
                                    tile_position=(h * 64, 0))
                                if chain is not None:
                                    tile.add_dep_helper(
                                        mm.ins, chain.ins, sync=False,
                                        reason="row-group pair adjacency")
                                chain = mm
                        for h in (0, 1):
                            if h not in pa:
                                continue
                            e = expp.tile([128, 1024], BF16, tag=f"e{h}", bufs=3,
                                          name=f"e{pair}{qh}{h}_{ktq[h]}")
                            nc.scalar.activation(e[:, :], pa[h][:, :], AF.Exp,
                                                 scale=exp_scale)
                            if pend_av[h] is not None:
                                emit_av(h)
                            pend_av[h] = (ktq[h], e)
                    for h in (0, 1):
                        if pend_av[h] is not None:
                            emit_av(h)
                    # normalize: gather 4 sumexp rows -> one reciprocal
                    for h in range(2):
                        for c in range(2):
                            r0 = 64 * c + 32 * h
                            nc.vector.tensor_copy(se[r0:r0 + 1, :],
                                                  avp[h][c][64:65, :])
                    recip4 = tmp.tile([128, 512], BF16, tag="recip4",
                                      name=f"rc{pair}{qh}")
                    _c = RECIP_APPROX_FAST_CONSTS
                    nc.vector._custom_dve(RECIPROCAL_APPROX_FAST,
                                          out=recip4[:, :], in0=se[:, :],
                                          s0=_c["s0"], s1=_c["s1"],
                                          imm2=_c["imm2"])
                    pb2 = ps2.tile([128, 1024], F32, tag="sc0",
                                   name=f"nb{pair}{qh}")
                    for c in range(2):
                        nc.tensor.matmul(pb2[:, c * 512:(c + 1) * 512],
                                         sel[:, c, :], recip4[:, :],
                                         start=True, stop=True)
                        avs2 = tmp.tile([128, 512], BF16, tag="avs2",
                                        name=f"avs{pair}{qh}{c}")
                        for h in range(2):
                            nc.vector.tensor_copy(avs2[h * 64:(h + 1) * 64, :],
                                                  avp[h][c][0:64, :])
                        nc.vector.tensor_mul(
                            vmix[pair][:, q0 + c * 512:q0 + (c + 1) * 512],
                            avs2[:, :], pb2[:, c * 512:(c + 1) * 512])

        # ---------------- phase 3: out proj (transposed) ----------------
        ncopy = 0
        with tc.tile_pool(name="ps3", bufs=1, space="PSUM") as ps3:
            for dmc in range(8):
                pos = [ps3.tile([128, 512], F32, tag=f"po{i}", bufs=2,
                                name=f"po{dmc}_{i}") for i in range(4)]
                for t in range(2):
                    for s4 in range(4):
                        nc.tensor.matmul(
                            pos[s4][:, :],
                            wout[:, t, dmc * 128:(dmc + 1) * 128],
                            vmix[t][:, s4 * 512:(s4 + 1) * 512],
                            start=(t == 0), stop=(t == 1))
                for s4 in range(4):
                    o = outp.tile([128, 512], BF16, tag=f"o{s4}", name=f"o{dmc}_{s4}")
                    if ncopy % 2 == 0:
                        nc.scalar.activation(o[:, :], pos[s4][:, :], AF.Copy)
                    else:
                        nc.vector.tensor_copy(o[:, :], pos[s4][:, :])
                    dmaq = nc.sync if ncopy % 2 == 0 else nc.gpsimd
                    ncopy += 1
                    dmaq.dma_start(
                        out=out_d.ap()[dmc * 128:(dmc + 1) * 128,
                                       s4 * 512:(s4 + 1) * 512],
                        in_=o[:, :])

    nc.compile()
    return nc


def host_prep(x, pos, Wqkv, bqkv, Wout, bout, q_scale, k_scale):
    """Build per-core input maps + shared-table decision."""
    x = np.asarray(x, dtype=np.float32)
    pos = np.asarray(pos, dtype=np.float32).reshape(-1)
    Wqkv = np.asarray(Wqkv, dtype=np.float32)
    bqkv = np.asarray(bqkv, dtype=np.float32)
    Wout = np.asarray(Wout, dtype=np.float32)
    q_scale = np.asarray(q_scale, dtype=np.float32)
    k_scale = np.asarray(k_scale, dtype=np.float32)

    shared = bool(np.array_equal(q_scale, k_scale))
    exp_scale = (1.0 / np.sqrt(DH)) if shared else 1.0

    bf = ml_dtypes.bfloat16
    # rope base tables [128, S]
    i_of_p = (np.arange(128) % 64) // 2            # pair index
    sign = np.where(np.arange(128) % 2 == 0, 1.0, -1.0)
    omega = THETA ** (-np.arange(0, DH, 2, dtype=np.float64) / DH)  # [32]
    ang = pos[None, :].astype(np.float64) * omega[:, None]          # [32, S]
    cosb = np.cos(ang)[i_of_p, :]                  # [128, S]
    sinb = np.sin(ang)[i_of_p, :] * sign[:, None]

    def tables(scale_vec, extra):
        sv = np.tile(scale_vec, 2)                 # [128]
        svx = np.tile(scale_vec[np.arange(64) ^ 1], 2)
        cosT = (cosb * sv[:, None] * extra).astype(bf)
        sinT = (sinb * svx[:, None] * extra).astype(bf)
        return np.ascontiguousarray(cosT), np.ascontiguousarray(sinT)

    cos_k, sin_k = tables(k_scale, 1.0)
    if not shared:
        cos_q, sin_q = tables(q_scale, 1.0 / np.sqrt(DH))

    Pm = np.zeros((128, 128), dtype=np.float32)
    Pm[np.arange(128), np.arange(128) ^ 1] = 1.0
    onesblk = np.zeros((128, 2), dtype=np.float32)
    onesblk[0:64, 0] = 1.0 / 128.0      # m' = 0.5 * mean(q^2)
    onesblk[64:128, 1] = 1.0 / 128.0
    ones2blk = np.zeros((2, 128), dtype=np.float32)
    ones2blk[0, 0:64] = 1.0
    ones2blk[1, 64:128] = 1.0
    # sel[:, v, :]: broadcast reciprocal row (h, v) to partitions h*64..
    sel = np.zeros((128, 2, 128), dtype=np.float32)
    for v in range(2):
        for h in range(2):
            sel[64 * v + 32 * h, v, h * 64:(h + 1) * 64] = 1.0

    in_maps = []
    for c in range(NC):
        b, g = c // 4, c % 4
        xT = np.ascontiguousarray(
            x[b].T.reshape(NDT, 128, S).transpose(1, 0, 2)).astype(bf)
        wq = Wqkv[:, g * HD:(g + 1) * HD]
        wk = Wqkv[:, DM + g * HD: DM + (g + 1) * HD]
        wv = Wqkv[:, 2 * DM + g * HD: 2 * DM + (g + 1) * HD]
        w_all = np.ascontiguousarray(
            np.concatenate([wq, wk, wv], axis=1)
            .reshape(NDT, 128, 3 * HD).transpose(1, 0, 2)).astype(bf)
        wo = np.ascontiguousarray(
            Wout[g * HD:(g + 1) * HD, :]
            .reshape(2, 128, DM).transpose(1, 0, 2)).astype(bf)
        bqs = np.ascontiguousarray(
            bqkv[g * HD:(g + 1) * HD].reshape(2, 128).T)         # [128, 2]
        bks = np.ascontiguousarray(
            bqkv[DM + g * HD: DM + (g + 1) * HD].reshape(2, 128).T)
        m = {"xT": xT, "w_all": w_all, "wout": wo, "bq": bqs, "bk": bks,
             "cos_k": cos_k, "sin_k": sin_k, "Pswap": Pm.astype(bf),
             "onesblk": onesblk.astype(bf), "ones2blk": ones2blk.astype(bf),
             "sel": sel.astype(bf)}
        if not shared:
            m["cos_q"] = cos_q
            m["sin_q"] = sin_q
        in_maps.append(m)

    bias_row = (bqkv[2 * DM:] @ Wout + np.asarray(bout, dtype=np.float32)) \
        .astype(np.float32)                                       # [1024]
    return in_maps, shared, float(exp_scale), bias_row


def _install_ntff_shim():
    """Make trace=True usable: this image lacks antenv.axon_hooks; recreate
    it against the baked libaxon_pjrt.so C ABI (no-op if already present)."""
    try:
        from antenv.axon_hooks import get_axon_ntff_profile_hook  # noqa: F401
        return
    except ImportError:
        pass
    try:
        import types, ctypes, contextlib
        import antenv
        lib = ctypes.CDLL("/opt/axon/libaxon_pjrt.so")
        if not hasattr(lib, "axon_start_nrt_profile"):
            raise OSError("no profile symbols")
        lib.axon_start_nrt_profile.argtypes = [ctypes.POINTER(ctypes.c_int64),
                                               ctypes.c_size_t]
        lib.axon_start_nrt_profile.restype = ctypes.c_int64
        lib.axon_stop_nrt_profile.argtypes = [ctypes.c_char_p]
        lib.axon_stop_nrt_profile.restype = ctypes.c_int64

        @contextlib.contextmanager
        def _hook(output_dir, device_ids):
            import jax
            jax.devices()
            if device_ids:
                ids = (ctypes.c_int64 * len(device_ids))(*device_ids)
                rc = lib.axon_start_nrt_profile(ids, len(device_ids))
            else:
                rc = lib.axon_start_nrt_profile(None, 0)
            if rc != 0:
                raise RuntimeError(f"axon_start_nrt_profile rc={rc}")
            try:
                yield
            finally:
                lib.axon_stop_nrt_profile(str(output_dir).encode())

        mod = types.ModuleType("antenv.axon_hooks")
        mod.get_axon_ntff_profile_hook = lambda: _hook
        mod.set_axon_ntff_profile_hook = lambda h: None
        sys.modules["antenv.axon_hooks"] = mod
        antenv.axon_hooks = mod
    except Exception:
        os.environ["BASS_NEVER_TRACE"] = "1"   # degrade: run untraced


def kernel(x, pos, Wqkv, bqkv, Wout, bout, q_scale, k_scale):
    global LAST_RESULTS
    if os.environ.get("BASS_TRACE"):
        _install_ntff_shim()
    in_maps, shared, exp_scale, bias_row = host_prep(
        x, pos, Wqkv, bqkv, Wout, bout, q_scale, k_scale)

    key = (shared, round(exp_scale, 9))
    if key not in _CACHED:
        _CACHED[key] = build_program(exp_scale, shared)
    nc = _CACHED[key]

    res = bass_utils.run_bass_kernel_spmd(
        nc, in_maps, list(range(NC)),
        trace=bool(os.environ.get("BASS_TRACE")))
    LAST_RESULTS = res

    out = np.empty((B, S, DM), dtype=np.float32)
    for b in range(B):
        acc = bias_row[None, :].astype(np.float32).repeat(S, axis=0)
        for g in range(4):
            acc = acc + res.results[b * 4 + g]["outp"].astype(np.float32).T
        out[b] = acc
    return out


# revision 13
# speedup vs baseline: 1.2506x; 1.2506x over previous
"""Trainium2 Bass kernel for nn_Attention_32650341384246.

Full attention layer: qkv proj + per-head RMSNorm(q,k) + RoPE + softmax
attention (non-causal) + out proj.  B=2, S=2048, D=1024, H=16, DH=64.

Sharding: 8 cores; core c handles batch c//4, heads [4*(c%4), 4*(c%4)+4)
(data parallel over batch x tensor parallel over heads).  Each core
computes a partial output (its heads @ Wout row-slice) TRANSPOSED
[DM, S]; the host sums the 4 partials per batch and adds folded biases.

Device design (per core), tuned for engine balance
(PE ~saturated; ACT carries all softmax exp; DVE the elementwise rest):
  - qkv proj emits qT/kT head-major [128 (2 heads x 64), S] (lhsT = W
    slice, rhs = xT slice) and v s-major [s, 4*64].
  - bias-add (tensor_scalar) + square (custom SQBIAS) on DVE; sumsq via
    ones-block matmul (bf16); rsqrt via 2-inst custom DVE chain
    (deg-3 Horner seed + fitted Newton step) -- no ACT Ln/Exp, so the
    whole kernel uses ONE activation table set (exp_and_others).
  - RoPE as q_rot = cosT*u + sinT'*swap(u); swap = adjacent-partition
    permutation matmul (bf16, exact); cos/sin tables bf16 with
    q_scale/k_scale folded in.
  - scores^T [k, q] bf16 matmuls per 512-wide q chunk (K=64), two heads
    on distinct PE row groups with forced emission adjacency so they run
    CONCURRENTLY; score PSUM double-buffered so sc(kt+1) overlaps exp(kt).
  - softmax exp entirely on ACT (PSUM -> bf16), 1/sqrt(dh) via exp scale.
  - AV via lhsT = [v | ones] bf16 (M=65): row 64 accumulates sumexp;
    h-outer/qv-inner order shares LDWEIGHTS between consecutive MMs.
  - normalize: gather 4 sumexp rows -> one DVE reciprocal -> bf16
    select-matrix matmul broadcasts reciprocal rows across partitions.
  - out proj TRANSPOSED: lhsT = Wout chunk (stationary across 4 s-chunks
    -> LDW amortized), rhs = vmix; psum -> bf16 copies -> DMA [DM, S].
"""
import sys, os

sys.path.insert(0, "/opt/trn_rl_repo")

import numpy as np
from contextlib import ExitStack

import ml_dtypes
import concourse.bass as bass
import concourse.mybir as mybir
import concourse.tile as tile
from concourse import bacc
from concourse import bass_utils
import concourse.dve_ops as dve_ops
from concourse.dve_ops import (DveOp, RECIPROCAL_APPROX_FAST,
                               RECIP_APPROX_FAST_CONSTS)
from concourse.dve_spec import (
    Spec, Src0, Src1, C0, C1, C2, C3, lower, _spill_c3_to_src1,
    _has_src1 as _has_src1,
)
from concourse.dve_uop import DveOpSpec

F32 = mybir.dt.float32
F32R = mybir.dt.float32r
BF16 = mybir.dt.bfloat16
AF = mybir.ActivationFunctionType

B, S, DM, H, DH = 2, 2048, 1024, 16, 64
NC = 8
HPC = H // 4          # 4 heads per core
HD = HPC * DH         # 256
NDT = DM // 128       # 8 model-dim tiles
THETA, EPS = 10000.0, 1e-6

LAST_RESULTS = None   # BassKernelResults of the most recent device run
_CACHED = {}

# knobs
T1_ON_GPSIMD = True    # u*cos and t1+t2 (SBUF-only) on GpSimd
PO_COPY_ACT = 16       # of 32 phase-3 psum->sbuf copies on ACT (rest DVE)

# ---------------- custom DVE ops (registered at import) ----------------


def _register_dve_op(name, spec, subdim=False):
    if name in dve_ops._SUB_OPCODE_FOR_NAME:
        for op in dve_ops.OPS:
            if op.name == name:
                return op
        raise RuntimeError(f"{name} registered but not in OPS")
    row = dve_ops._CUSTOM_DVE_ROW_BASE + len(dve_ops.OPS)
    assert row < 0x20, "custom DVE op rows exhausted"
    dve_ops._SUB_OPCODE_FOR_NAME[name] = row
    shas = {"v3": DveOpSpec(name=name, opcode=row,
                            uops=lower(spec, ver="v3"),
                            rd1_en=_has_src1(spec)).sha("v3")}
    op = DveOp(name, spec, subdim=subdim, uops_sha=shas)
    dve_ops.OPS.append(op)
    dve_ops.CUSTOM_DVE_SPECS[name] = spec
    return op


# sq = (x + b)^2 with per-partition b; reads score PSUM once.
_sb = Src0 + C0
SQBIAS = _register_dve_op(
    "SQBIAS_ANT",
    Spec(body=_sb * _sb,
         reference=lambda in0, in1, s0, s1, imm2: (
             (np.asarray(in0, np.float32) + np.float32(s0)) ** 2
         ).astype(np.float32)))

# rsqrt(2m) over m in [0.052, 0.385]: deg-3 Horner seed ...
RSQ_C3 = -88.55851031561393
RSQ_C2 = 78.57457530349905
RSQ_C1 = -24.768702251743473
RSQ_C0 = 4.054988803119327   # via in1 [P,1]
_s1 = Src0 * C0
_s2 = _s1 + C1
_s3 = _s2 * Src0
_s4 = _s3 + C2
_s5 = _s4 * Src0
_seed_body = _spill_c3_to_src1(_s5 + C3)


def _ref_rsq_seed(in0, in1, s0, s1, imm2):
    m = np.asarray(in0, np.float32)
    c0 = np.asarray(in1, np.float32).reshape(m.shape[0], 1)
    t = (((m * np.float32(s0) + np.float32(s1)) * m + np.float32(imm2)) * m
         + c0)
    return t.astype(np.float32)


RSQ_SEED = _register_dve_op("RSQ_SEED_ANT",
                            Spec(body=_seed_body, reference=_ref_rsq_seed))

# ... then fitted Newton y1 = q*(A - B*m*q^2), q via in1.
RSQ_NA = 1.518420851483698
RSQ_NB = 1.035705175407688
_q2 = Src1 * Src1
_h = _q2 * Src0
_hb = _h * C0
_w = C1 - _hb
_newt_body = Src1 * _w


def _ref_rsq_newt(in0, in1, s0, s1, imm2):
    m = np.asarray(in0, np.float32)
    q = np.asarray(in1, np.float32)
    w = (np.float32(s1) - q * q * m * np.float32(s0)).astype(np.float32)
    return (q * w).astype(np.float32)


RSQ_NEWT = _register_dve_op("RSQ_NEWT_ANT",
                            Spec(body=_newt_body, reference=_ref_rsq_newt))


def build_program(exp_scale: float, shared_tables: bool):
    nc = bacc.Bacc("TRN2", target_bir_lowering=False, debug=False)

    xT_d = nc.dram_tensor("xT", [128, NDT, S], BF16, kind="ExternalInput")
    w_d = nc.dram_tensor("w_all", [128, NDT, 3 * HD], BF16, kind="ExternalInput")
    wout_d = nc.dram_tensor("wout", [128, 2, DM], BF16, kind="ExternalInput")
    bq_d = nc.dram_tensor("bq", [128, 2], F32, kind="ExternalInput")
    bk_d = nc.dram_tensor("bk", [128, 2], F32, kind="ExternalInput")
    cosk_d = nc.dram_tensor("cos_k", [128, S], BF16, kind="ExternalInput")
    sink_d = nc.dram_tensor("sin_k", [128, S], BF16, kind="ExternalInput")
    if not shared_tables:
        cosq_d = nc.dram_tensor("cos_q", [128, S], BF16, kind="ExternalInput")
        sinq_d = nc.dram_tensor("sin_q", [128, S], BF16, kind="ExternalInput")
    P_d = nc.dram_tensor("Pswap", [128, 128], BF16, kind="ExternalInput")
    ob_d = nc.dram_tensor("onesblk", [128, 2], BF16, kind="ExternalInput")
    o2_d = nc.dram_tensor("ones2blk", [2, 128], BF16, kind="ExternalInput")
    sel_d = nc.dram_tensor("sel", [128, 2, 128], BF16, kind="ExternalInput")
    out_d = nc.dram_tensor("outp", [DM, S], BF16, kind="ExternalOutput")

    with tile.TileContext(nc) as tc, ExitStack() as ctx, \
            nc.allow_low_precision(reason="fp32r/bf16 matmul inputs"):
        singles = ctx.enter_context(tc.tile_pool(name="singles", bufs=1))
        tmp = ctx.enter_context(tc.tile_pool(name="tmp", bufs=2))
        expp = ctx.enter_context(tc.tile_pool(name="expp", bufs=2))
        outp = ctx.enter_context(tc.tile_pool(name="outp", bufs=2))

        w_dt = [singles.tile([128, 3 * HD], BF16, name=f"w{dt}") for dt in range(NDT)]
        x_dt = [singles.tile([128, S], BF16, name=f"x{dt}") for dt in range(NDT)]
        for dt in range(NDT):
            nc.sync.dma_start(out=w_dt[dt], in_=w_d.ap()[:, dt, :])
            nc.sync.dma_start(out=x_dt[dt], in_=xT_d.ap()[:, dt, :])

        wout = singles.tile([128, 2, DM], BF16)
        nc.sync.dma_start(out=wout, in_=wout_d.ap())
        bq = singles.tile([128, 2], F32)
        nc.sync.dma_start(out=bq, in_=bq_d.ap())
        bk = singles.tile([128, 2], F32)
        nc.sync.dma_start(out=bk, in_=bk_d.ap())
        cos_k = singles.tile([128, S], BF16)
        nc.sync.dma_start(out=cos_k, in_=cosk_d.ap())
        sin_k = singles.tile([128, S], BF16)
        nc.sync.dma_start(out=sin_k, in_=sink_d.ap())
        if shared_tables:
            cos_q, sin_q = cos_k, sin_k
        else:
            cos_q = singles.tile([128, S], BF16)
            nc.sync.dma_start(out=cos_q, in_=cosq_d.ap())
            sin_q = singles.tile([128, S], BF16)
            nc.sync.dma_start(out=sin_q, in_=sinq_d.ap())
        Pm = singles.tile([128, 128], BF16)
        nc.sync.dma_start(out=Pm, in_=P_d.ap())
        onesblk = singles.tile([128, 2], BF16)
        nc.sync.dma_start(out=onesblk, in_=ob_d.ap())
        ones2blk = singles.tile([2, 128], BF16)
        nc.sync.dma_start(out=ones2blk, in_=o2_d.ap())
        sel = singles.tile([128, 2, 128], BF16)
        nc.sync.dma_start(out=sel, in_=sel_d.ap())
        c0t = singles.tile([128, 1], F32)
        nc.vector.memset(c0t, RSQ_C0)

        qt = [singles.tile([128, S], BF16, name=f"qt{t}") for t in range(2)]
        kt_ = [singles.tile([128, S], BF16, name=f"kt{t}") for t in range(2)]
        vhat = singles.tile([128, 16, HPC, 65], BF16, name="vhat")
        nc.vector.memset(vhat[:, :, :, 64:65], 1.0)
        vmix = [singles.tile([128, S], BF16, name=f"vmix{t}") for t in range(2)]
        se = singles.tile([128, 512], F32, name="se")
        nc.vector.memset(se, 1.0)

        # ---------------- phase 1: qkv + rmsnorm + rope ----------------
        with tc.tile_pool(name="ps1", bufs=1, space="PSUM") as ps1:
            sections = (
                    ("k", 0, bk, cos_k, sin_k, kt_),
                    ("q", 0, bq, cos_q, sin_q, qt),
                    ("k", 1, bk, cos_k, sin_k, kt_),
                    ("q", 1, bq, cos_q, sin_q, qt))
            for which, t, bias, cosT, sinT, dest in sections:
                off = (0 if which == "q" else HD) + t * 128
                for sc in range(4):       # s-chunks of 512
                    s0 = sc * 512
                    pq = ps1.tile([128, 512], F32, tag="pq", bufs=2,
                                  name=f"pq{which}{t}_{sc}")
                    for dt in range(NDT):
                        nc.tensor.matmul(
                            pq[:, :],
                            w_dt[dt][:, off: off + 128],
                            x_dt[dt][:, s0:s0 + 512],
                            start=(dt == 0), stop=(dt == NDT - 1))
                    tt = tmp.tile([128, 512], F32, tag="tt", bufs=4,
                                  name=f"tt{which}{t}_{sc}")
                    nc.scalar.activation(tt[:, :], pq[:, :], AF.Identity,
                                         bias=bias[:, t:t + 1], scale=1.0)
                    sq = tmp.tile([128, 512], BF16, tag="sq", name=f"sq{which}{t}_{sc}")
                    nc.scalar.activation(sq[:, :], pq[:, :], AF.Square,
                                         bias=bias[:, t:t + 1], scale=1.0)
                    pss = ps1.tile([2, 512], F32, tag="pss", bufs=2,
                                   name=f"pss{which}{t}_{sc}")
                    nc.tensor.matmul(pss[:, :], onesblk[:, :], sq[:, :],
                                     start=True, stop=True)
                    seed = tmp.tile([2, 512], F32, tag="seed", name=f"sd{which}{t}_{sc}")
                    nc.vector._custom_dve(RSQ_SEED, out=seed[:, :], in0=pss[:, :],
                                          in1=c0t[0:2, 0:1],
                                          s0=RSQ_C3, s1=RSQ_C2, imm2=RSQ_C1)
                    rs = tmp.tile([2, 512], BF16, tag="rs", name=f"rs{which}{t}_{sc}")
                    nc.vector._custom_dve(RSQ_NEWT, out=rs[:, :], in0=pss[:, :],
                                          in1=seed[:, :],
                                          s0=RSQ_NB, s1=RSQ_NA)
                    pb = ps1.tile([128, 512], F32, tag="pb",
                                  name=f"pb{which}{t}_{sc}")
                    nc.tensor.matmul(pb[:, :], ones2blk[:, :], rs[:, :],
                                     start=True, stop=True)
                    u = tmp.tile([128, 512], BF16, tag="u", name=f"u{which}{t}_{sc}")
                    nc.vector.tensor_mul(u[:, :], tt[:, :], pb[:, :])
                    psw = ps1.tile([128, 512], F32, tag="psw",
                                   name=f"psw{which}{t}_{sc}")
                    nc.tensor.matmul(psw[:, :], Pm[:, :], u[:, :],
                                     start=True, stop=True)
                    t1 = tmp.tile([128, 512], BF16, tag="t1", name=f"t1{which}{t}_{sc}")
                    eng1 = nc.gpsimd if T1_ON_GPSIMD else nc.vector
                    eng1.tensor_mul(t1[:, :], u[:, :], cosT[:, s0:s0 + 512])
                    t2 = tmp.tile([128, 512], BF16, tag="t2", name=f"t2{which}{t}_{sc}")
                    nc.vector.tensor_mul(t2[:, :], psw[:, :], sinT[:, s0:s0 + 512])
                    eng1.tensor_add(dest[t][:, s0:s0 + 512], t1[:, :], t2[:, :])

            # v section: s-major [s, 4*64] + ones column
            for kt in range(16):
                pv = ps1.tile([128, HD], F32, tag="pv", bufs=2, name=f"pv{kt}")
                for dt in range(NDT):
                    nc.tensor.matmul(
                        pv[:, :],
                        x_dt[dt][:, kt * 128: (kt + 1) * 128],
                        w_dt[dt][:, 2 * HD:3 * HD],
                        start=(dt == 0), stop=(dt == NDT - 1))
                nc.vector.tensor_copy(vhat[:, kt, :, 0:64],
                                      pv[:, :].rearrange("p (h d) -> p h d", h=HPC))

        # ---------------- phase 2: attention ----------------
        # Proven structure: per (pair, q-half) group, per kt: 4 score MMs
        # (2 heads on distinct PE row groups x 2 q-chunks), one [128,1024]
        # exp per head on ACT, then 4 AV MMs.  Score PSUM is single-
        # buffered per head; deep es buffering (bufs=4) keeps ACT dense.
        with tc.tile_pool(name="ps2", bufs=1, space="PSUM") as ps2:
            for pair in range(2):
                for qh in range(2):
                    q0 = qh * 1024
                    ps_sc = [ps2.tile([128, 1024], F32, tag=f"sc{h}",
                                      name=f"sc{pair}{qh}{h}") for h in range(2)]
                    avp = [[ps2.tile([65, 512], F32, tag=f"av{h}{c}",
                                     name=f"av{pair}{qh}{h}{c}")
                            for c in range(2)] for h in range(2)]
                    for kt in range(16):
                        for c in range(2):
                            for h in range(2):
                                nc.tensor.matmul(
                                    ps_sc[h][:, c * 512:(c + 1) * 512],
                                    kt_[pair][h * 64:(h + 1) * 64,
                                              kt * 128:(kt + 1) * 128],
                                    qt[pair][h * 64:(h + 1) * 64,
                                             q0 + c * 512:q0 + (c + 1) * 512],
                                    start=True, stop=True,
                                    tile_position=(h * 64, 0))
                        es = []
                        for h in range(2):
                            e = expp.tile([128, 1024], BF16, tag=f"e{h}", bufs=4,
                                          name=f"e{pair}{qh}{h}_{kt}")
                            nc.scalar.activation(e[:, :], ps_sc[h][:, :], AF.Exp,
                                                 scale=exp_scale)
                            es.append(e)
                        for h in range(2):
                            head = 2 * pair + h
                            for c in range(2):
                                nc.tensor.matmul(
                                    avp[h][c][:, :],
                                    vhat[:, kt, head, :],
                                    es[h][:, c * 512:(c + 1) * 512],
                                    start=(kt == 0), stop=(kt == 15),
                                    skip_group_check=True)
                    # normalize: gather 4 sumexp rows -> one reciprocal
                    for h in range(2):
                        for c in range(2):
                            r0 = 64 * c + 32 * h
                            nc.vector.tensor_copy(se[r0:r0 + 1, :],
                                                  avp[h][c][64:65, :])
                    recip4 = tmp.tile([128, 512], BF16, tag="recip4",
                                      name=f"rc{pair}{qh}")
                    _c = RECIP_APPROX_FAST_CONSTS
                    nc.vector._custom_dve(RECIPROCAL_APPROX_FAST,
                                          out=recip4[:, :], in0=se[:, :],
                                          s0=_c["s0"], s1=_c["s1"],
                                          imm2=_c["imm2"])
                    for c in range(2):
                        pb2 = ps2.tile([128, 512], F32, tag=f"av0{c}",
                                       name=f"nb{pair}{qh}{c}")
                        nc.tensor.matmul(pb2[:, :], sel[:, c, :], recip4[:, :],
                                         start=True, stop=True)
                        avs2 = tmp.tile([128, 512], BF16, tag="avs2",
                                        name=f"avs{pair}{qh}{c}")
                        for h in range(2):
                            nc.vector.tensor_copy(avs2[h * 64:(h + 1) * 64, :],
                                                  avp[h][c][0:64, :])
                        nc.vector.tensor_mul(
                            vmix[pair][:, q0 + c * 512:q0 + (c + 1) * 512],
                            avs2[:, :], pb2[:, :])

        # ---------------- phase 3: out proj (transposed) ----------------
        ncopy = 0
        with tc.tile_pool(name="ps3", bufs=1, space="PSUM") as ps3:
            for dmc in range(8):
                pos = [ps3.tile([128, 512], F32, tag=f"po{i}", bufs=2,
                                name=f"po{dmc}_{i}") for i in range(4)]
                for t in range(2):
                    for s4 in range(4):
                        nc.tensor.matmul(
                            pos[s4][:, :],
                            wout[:, t, dmc * 128:(dmc + 1) * 128],
                            vmix[t][:, s4 * 512:(s4 + 1) * 512],
                            start=(t == 0), stop=(t == 1))
                for s4 in range(4):
                    o = outp.tile([128, 512], BF16, tag=f"o{s4}", name=f"o{dmc}_{s4}")
                    if ncopy % 2 == 0:
                        nc.scalar.activation(o[:, :], pos[s4][:, :], AF.Copy)
                    else:
                        nc.vector.tensor_copy(o[:, :], pos[s4][:, :])
                    dmaq = nc.sync if ncopy % 2 == 0 else nc.gpsimd
                    ncopy += 1
                    dmaq.dma_start(
                        out=out_d.ap()[dmc * 128:(dmc + 1) * 128,
                                       s4 * 512:(s4 + 1) * 512],
                        in_=o[:, :])

    nc.compile()
    return nc


def host_prep(x, pos, Wqkv, bqkv, Wout, bout, q_scale, k_scale):
    """Build per-core input maps + shared-table decision."""
    x = np.asarray(x, dtype=np.float32)
    pos = np.asarray(pos, dtype=np.float32).reshape(-1)
    Wqkv = np.asarray(Wqkv, dtype=np.float32)
    bqkv = np.asarray(bqkv, dtype=np.float32)
    Wout = np.asarray(Wout, dtype=np.float32)
    q_scale = np.asarray(q_scale, dtype=np.float32)
    k_scale = np.asarray(k_scale, dtype=np.float32)

    shared = bool(np.array_equal(q_scale, k_scale))
    exp_scale = (1.0 / np.sqrt(DH)) if shared else 1.0

    bf = ml_dtypes.bfloat16
    # rope base tables [128, S]
    i_of_p = (np.arange(128) % 64) // 2            # pair index
    sign = np.where(np.arange(128) % 2 == 0, 1.0, -1.0)
    omega = THETA ** (-np.arange(0, DH, 2, dtype=np.float64) / DH)  # [32]
    ang = pos[None, :].astype(np.float64) * omega[:, None]          # [32, S]
    cosb = np.cos(ang)[i_of_p, :]                  # [128, S]
    sinb = np.sin(ang)[i_of_p, :] * sign[:, None]

    def tables(scale_vec, extra):
        sv = np.tile(scale_vec, 2)                 # [128]
        svx = np.tile(scale_vec[np.arange(64) ^ 1], 2)
        cosT = (cosb * sv[:, None] * extra).astype(bf)
        sinT = (sinb * svx[:, None] * extra).astype(bf)
        return np.ascontiguousarray(cosT), np.ascontiguousarray(sinT)

    cos_k, sin_k = tables(k_scale, 1.0)
    if not shared:
        cos_q, sin_q = tables(q_scale, 1.0 / np.sqrt(DH))

    Pm = np.zeros((128, 128), dtype=np.float32)
    Pm[np.arange(128), np.arange(128) ^ 1] = 1.0
    onesblk = np.zeros((128, 2), dtype=np.float32)
    onesblk[0:64, 0] = 1.0 / 128.0      # m' = 0.5 * mean(q^2)
    onesblk[64:128, 1] = 1.0 / 128.0
    ones2blk = np.zeros((2, 128), dtype=np.float32)
    ones2blk[0, 0:64] = 1.0
    ones2blk[1, 64:128] = 1.0
    # sel[:, v, :]: broadcast reciprocal row (h, v) to partitions h*64..
    sel = np.zeros((128, 2, 128), dtype=np.float32)
    for v in range(2):
        for h in range(2):
            sel[64 * v + 32 * h, v, h * 64:(h + 1) * 64] = 1.0

    in_maps = []
    for c in range(NC):
        b, g = c // 4, c % 4
        xT = np.ascontiguousarray(
            x[b].T.reshape(NDT, 128, S).transpose(1, 0, 2)).astype(bf)
        wq = Wqkv[:, g * HD:(g + 1) * HD]
        wk = Wqkv[:, DM + g * HD: DM + (g + 1) * HD]
        wv = Wqkv[:, 2 * DM + g * HD: 2 * DM + (g + 1) * HD]
        w_all = np.ascontiguousarray(
            np.concatenate([wq, wk, wv], axis=1)
            .reshape(NDT, 128, 3 * HD).transpose(1, 0, 2)).astype(bf)
        wo = np.ascontiguousarray(
            Wout[g * HD:(g + 1) * HD, :]
            .reshape(2, 128, DM).transpose(1, 0, 2)).astype(bf)
        bqs = np.ascontiguousarray(
            bqkv[g * HD:(g + 1) * HD].reshape(2, 128).T)         # [128, 2]
        bks = np.ascontiguousarray(
            bqkv[DM + g * HD: DM + (g + 1) * HD].reshape(2, 128).T)
        m = {"xT": xT, "w_all": w_all, "wout": wo, "bq": bqs, "bk": bks,
             "cos_k": cos_k, "sin_k": sin_k, "Pswap": Pm.astype(bf),
             "onesblk": onesblk.astype(bf), "ones2blk": ones2blk.astype(bf),
             "sel": sel.astype(bf)}
        if not shared:
            m["cos_q"] = cos_q
            m["sin_q"] = sin_q
        in_maps.append(m)

    bias_row = (bqkv[2 * DM:] @ Wout + np.asarray(bout, dtype=np.float32)) \
        .astype(np.float32)                                       # [1024]
    return in_maps, shared, float(exp_scale), bias_row


def _install_ntff_shim():
    """Make trace=True usable: this image lacks antenv.axon_hooks; recreate
    it against the baked libaxon_pjrt.so C ABI (no-op if already present)."""
    try:
        from antenv.axon_hooks import get_axon_ntff_profile_hook  # noqa: F401
        return
    except ImportError:
        pass
    try:
        import types, ctypes, contextlib
        import antenv
        lib = ctypes.CDLL("/opt/axon/libaxon_pjrt.so")
        if not hasattr(lib, "axon_start_nrt_profile"):
            raise OSError("no profile symbols")
        lib.axon_start_nrt_profile.argtypes = [ctypes.POINTER(ctypes.c_int64),
                                               ctypes.c_size_t]
        lib.axon_start_nrt_profile.restype = ctypes.c_int64
        lib.axon_stop_nrt_profile.argtypes = [ctypes.c_char_p]
        lib.axon_stop_nrt_profile.restype = ctypes.c_int64

        @contextlib.contextmanager
        def _hook(output_dir, device_ids):
            import jax
            jax.devices()
            if device_ids:
                ids = (ctypes.c_int64 * len(device_ids))(*device_ids)
                rc = lib.axon_start_nrt_profile(ids, len(device_ids))
            else:
                rc = lib.axon_start_nrt_profile(None, 0)
            if rc != 0:
                raise RuntimeError(f"axon_start_nrt_profile rc={rc}")
            try:
                yield
            finally:
                lib.axon_stop_nrt_profile(str(output_dir).encode())

        mod = types.ModuleType("antenv.axon_hooks")
        mod.get_axon_ntff_profile_hook = lambda: _hook
        mod.set_axon_ntff_profile_hook = lambda h: None
        sys.modules["antenv.axon_hooks"] = mod
        antenv.axon_hooks = mod
    except Exception:
        os.environ["BASS_NEVER_TRACE"] = "1"   # degrade: run untraced


def kernel(x, pos, Wqkv, bqkv, Wout, bout, q_scale, k_scale):
    global LAST_RESULTS
    if os.environ.get("BASS_TRACE"):
        _install_ntff_shim()
    in_maps, shared, exp_scale, bias_row = host_prep(
        x, pos, Wqkv, bqkv, Wout, bout, q_scale, k_scale)

    key = (shared, round(exp_scale, 9))
    if key not in _CACHED:
        _CACHED[key] = build_program(exp_scale, shared)
    nc = _CACHED[key]

    res = bass_utils.run_bass_kernel_spmd(
        nc, in_maps, list(range(NC)),
        trace=bool(os.environ.get("BASS_TRACE")))
    LAST_RESULTS = res

    out = np.empty((B, S, DM), dtype=np.float32)
    for b in range(B):
        acc = bias_row[None, :].astype(np.float32).repeat(S, axis=0)
        for g in range(4):
            acc = acc + res.results[b * 4 + g]["outp"].astype(np.float32).T
        out[b] = acc
    return out
